# revision 2
# baseline (speedup 1.0000x reference)
"""Per-pixel predicted 5x5 conv (KPN-style) on 8 trn2 cores — banded-matmul PE design.

Sharding: data-parallel over (batch x H-half) = 8 shards of 128 output rows.

All 25 tap-multiplies run on the PE as banded matmuls contracting over w'
(the source column):
    out[h, w, c] = sum_{i,j} kern[h,w,i*5+j] * feat[h+i-2, w+j-2, c]
For one feat row F = h0+rt-2 and one 128-wide w-tile (4 groups of 32), a
single K=116 matmul computes the j-sums for 16 w-offsets x 5 output rows
(h = rt-4..rt) x 4 pixel-groups x 32 channels:
  - stationary lhsT [116, 128]: partition rows [32g, 32g+20) hold
    feat[F, 32g+base+k, c] in a block-diagonal 128-wide layout (g-block at
    columns 32g..32g+32, zeros elsewhere); built by partition-aligned DVE
    copies into a double-buffered generation arena from a per-(wt,half)
    feat upload; the zero regions are memset once and persist.
  - moving rhs [116, 16x5]: kernel taps, row 32g+w16+j of column (w16, hidx)
    holds kern[h, w, i*5+j] with i = 4-hidx; resident class arena
    [128, (cls 32, t 264, hidx 5)] with cls = 2*w16+half, uploaded by 32
    composite-partition DMAs; in-range off-band rows zeroed once (split
    across DVE/ACT/Pool).
  - psum [128=(g,c), (w-off 32, hslot 32)] accumulates each row's 5 i-visits;
    start=True on the first visit (hidx=4) replaces bias seeds.
  - ACT evacuates 16-row batches psum->SBUF fp16 with the channel bias folded
    in via the per-partition activation bias operand.
Output leaves in the native [128=(g,c), wt, h, w-off] layout; the host
unshards.
"""

import sys

for p in ("/opt/pypackages", "/opt/trn_rl_repo"):
    if p not in sys.path:
        sys.path.insert(0, p)

import numpy as np

import concourse.mybir as mybir
from concourse import bacc, tile
from concourse.bass_utils import run_bass_kernel_spmd

B, H, W, C, KK, K = 4, 256, 256, 32, 25, 5
HS = H // 2            # 128 output rows per core
NG, GW = 4, 32         # 4 w-groups of 32 per w-tile
NWT = 2                # two 128-wide w-tiles
NRT = HS + 4           # 132 stationary feat-row tiles (h0-2 .. h0+129)
GEN = 33               # r-slots per stationary generation
NGEN = NRT // GEN      # 4
RING = 32              # psum h-slot ring
EVB = 16               # evac h-batch
NCLS = 32              # moving classes; cls = 16*half + w16
NT = NRT * NWT         # 264 tiles per class
CLS_P = NT * 5         # class free-pitch in elements (1320)
MV_FD = NCLS * CLS_P   # moving arena free elems (42240)
FT_FD = NRT * C        # featT free elems (4224)
GB_FD = GEN * 128      # genbuf free elems (4224)
KDIM = 128             # contraction rows (4 ranges of 20 at 32-aligned bases, zero-padded)

F16 = mybir.dt.float16
F32 = mybir.dt.float32
U32 = mybir.dt.uint32

_NC_CACHE = {}


def _build_nc():
    nc = bacc.Bacc(None, target_bir_lowering=False)
    featt_d = nc.dram_tensor("featt", [4, 128, NRT, C], F16, kind="ExternalInput")
    mov_d = nc.dram_tensor("movz", [128, MV_FD], F16, kind="ExternalInput")
    bias_d = nc.dram_tensor("biasr", [128, 1], F32, kind="ExternalInput")
    brow_d = nc.dram_tensor("brow", [1, 128], F16, kind="ExternalInput")
    ones_d = nc.dram_tensor("ones", [1, 512], F16, kind="ExternalInput")
    out_d = nc.dram_tensor("outd", [128, NWT, HS, GW], F16, kind="ExternalOutput")

    with tile.TileContext(nc) as tc:
        with tc.tile_pool(name="const", bufs=1) as cpool, \
             tc.tile_pool(name="gen", bufs=1) as gpool, \
             tc.tile_pool(name="stg", bufs=1) as spool, \
             tc.tile_pool(name="psum", bufs=1, space="PSUM") as qpool:
            biast = cpool.tile([128, 1], F32, tag="biast")
            brow = cpool.tile([1, 128], F16, tag="brow")
            onest = cpool.tile([1, 512], F16, tag="onest")
            marena = cpool.tile([128, MV_FD], F16, tag="marena")
            featT = []
            for s in range(4):
                t = cpool.tile([128, FT_FD], F16, tag=f"ft{s}", name=f"featT_{s}")
                featT.append(t)
            genbuf = []
            for s in range(4):
                pair = []
                for pb in range(2):
                    t = gpool.tile([128, GB_FD], F16, tag=f"gb{s}_{pb}",
                                   name=f"genbuf_{s}_{pb}")
                    pair.append(t)
                genbuf.append(pair)
            psum = []
            for wt in range(NWT):
                t = qpool.tile([128, GW * RING], F32, tag=f"ps{wt}",
                               name=f"psum_{wt}")
                psum.append(t)
            stage = []
            for sb in range(2):
                t = spool.tile([128, NWT * EVB * GW], F16, tag=f"st{sb}",
                               name=f"stage_{sb}")
                stage.append(t)

            mv_v = marena.rearrange("p (c t h) -> p c t h", c=NCLS, t=NT)

            def zero_u32(eng, reg):
                ru = reg.bitcast(U32)
                if eng == "dve":
                    nc.vector.tensor_scalar_mul(ru, ru, 0.0)
                elif eng == "act":
                    nc.scalar.memzero(reg)
                else:
                    nc.gpsimd.memset(reg, 0.0)

            # ---------- phase 0 ----------
            nc.sync.dma_start(out=biast, in_=bias_d[:, :])
            nc.sync.dma_start(out=brow, in_=brow_d[:, :])
            nc.sync.dma_start(out=onest, in_=ones_d[:, :])
            # feat uploads: plain dense (zero gap rows prebuilt on host)
            for s in range(4):
                nc.sync.dma_start(
                    out=featT[s].rearrange("p (r c) -> p r c", r=NRT),
                    in_=featt_d[s])
            # genbuf pb0 zeros on DVE (gates gen-0 builds), then builds
            for s in range(4):
                zero_u32("dve", genbuf[s][0][:, :])

            def build_gen(gi):
                pb = gi % 2
                r0 = gi * GEN
                for s in range(4):
                    ft_v = featT[s].rearrange("p (r c) -> p r c", r=NRT)
                    gb_v = genbuf[s][pb].rearrange("p (r x) -> p r x", r=GEN)
                    for g in range(NG):
                        nc.vector.tensor_copy(
                            out=gb_v[32 * g:32 * g + 20, :, GW * g:GW * g + GW],
                            in_=ft_v[32 * g:32 * g + 20, r0:r0 + GEN, :])

            build_gen(0)

            # moving arena: zeros prebuilt on host, 4 big plain DMAs
            Q = 8 * CLS_P
            for ch in range(4):
                nc.sync.dma_start(out=marena[:, ch * Q:(ch + 1) * Q],
                                  in_=mov_d[:, ch * Q:(ch + 1) * Q])

            for s in range(4):
                zero_u32("dve", genbuf[s][1][:, :])
            build_gen(1)

            # ---------- main loop ----------
            ps_v = [psum[wt].rearrange("p (s w) -> p s w", s=RING)
                    for wt in range(NWT)]
            st_v = [stage[sb].rearrange("p (w h x) -> p w h x", w=NWT, h=EVB)
                    for sb in range(2)]

            def sub_mm(rt, wt, half, hidx):
                s = 2 * wt + half
                gi, slot = rt // GEN, rt % GEN
                lhsT = genbuf[s][gi % 2][0:KDIM, 128 * slot:128 * slot + 128]
                t = 2 * rt + wt
                rhs = mv_v[0:KDIM, 16 * half:16 * half + 16, t, hidx]
                sl = (rt - 4 + hidx) % RING
                out_ap = ps_v[wt][:, sl, 16 * half:16 * half + 16]
                nc.tensor.matmul(out_ap, lhsT, rhs,
                                 start=False, stop=False,
                                 skip_group_check=True, tile_position=(0, 0))

            def seed(hb, wt):
                # bias-seed one full psum bank (16 h-slots) with start=True:
                # start clears has_written bank-wide, so it must cover the
                # bank exactly, before any accumulation into it.
                b0 = ((hb % RING) // EVB) * 512
                nc.tensor.matmul(psum[wt][:, b0:b0 + 512], brow[0:1, :],
                                 onest[0:1, :], start=True, stop=False,
                                 skip_group_check=True, tile_position=(0, 0))

            def evac(hb, wt):
                sb = (hb // EVB) % 2
                sl0 = hb % RING
                src = ps_v[wt][:, sl0:sl0 + EVB, :]
                dst = st_v[sb][:, wt, :, :]
                nc.scalar.copy(out=dst, in_=src)

            def flush(hb):
                sb = (hb // EVB) % 2
                nc.sync.dma_start(
                    out=out_d[:, :, hb:hb + EVB, :], in_=st_v[sb])

            for rt in range(NRT):
                if rt % GEN == 0 and rt // GEN >= 1 and rt // GEN + 1 < NGEN:
                    build_gen(rt // GEN + 1)
                if rt % EVB == 0 and rt < HS:
                    for wt in range(NWT):
                        seed(rt, wt)
                for wt in range(NWT):
                    for half in range(2):
                        for hidx in range(5):
                            sub_mm(rt, wt, half, hidx)
                hb = rt - (EVB + 3)
                if hb >= 0 and hb % EVB == 0 and hb < HS:
                    for wt in range(NWT):
                        evac(hb, wt)
                    flush(hb)
    if not nc.is_finalized():
        nc.finalize()
    return nc


def _get_nc():
    if "nc" not in _NC_CACHE:
        _NC_CACHE["nc"] = _build_nc()
    return _NC_CACHE["nc"]


def _prep_inputs(feat, kernel, bias):
    feat16 = feat.astype(np.float16)
    k16 = kernel.astype(np.float16)
    in_maps = []
    fp = np.zeros((B, H + 4, W + 4, C), np.float16)
    fp[:, 2:H + 2, 2:W + 2, :] = feat16

    bias_rep = np.tile(bias.astype(np.float32), NG)[:, None]
    bias_rep = np.ascontiguousarray(bias_rep)

    cls_i = np.arange(NCLS)
    half_i = cls_i // 16
    w16_i = cls_i % 16
    g_i = np.arange(NG)
    j_i = np.arange(5)
    t_i = np.arange(NT)
    rt_i = t_i // 2
    wt_i = t_i % 2
    hx_i = np.arange(5)
    W_idx = (128 * wt_i[None, None, :] + 32 * g_i[None, :, None]
             + (16 * half_i + w16_i)[:, None, None])          # [cls, g, t]
    Hrel = rt_i[:, None] - 4 + hx_i[None, :]                   # [t, hidx]
    TAP = (4 - hx_i)[None, :] * 5 + j_i[:, None]               # [j, hidx]
    h_valid = (Hrel >= 0) & (Hrel < HS)

    for core in range(8):
        b, hh = core // 2, core % 2
        h0 = hh * HS

        featt = np.zeros((4, 128, NRT, C), np.float16)
        rows_h = h0 + np.arange(NRT)
        for wt in range(NWT):
            for half in range(2):
                s = 2 * wt + half
                base = 128 * wt + (-2 if half == 0 else 14)
                for g in range(NG):
                    wcols = base + 32 * g + np.arange(20) + 2
                    featt[s, 32 * g:32 * g + 20] = (
                        fp[b, rows_h][:, wcols, :].transpose(1, 0, 2))

        kc = k16[b, h0:h0 + HS]
        hcl = np.clip(Hrel, 0, HS - 1)
        movv = kc[hcl[None, None, None, :, :],
                  W_idx[:, :, None, :, None],
                  TAP[None, None, :, None, :]]
        movv = (movv * h_valid[None, None, None, :, :]).astype(np.float16)
        movz = np.zeros((128, NCLS, NT, 5), np.float16)
        for g in range(NG):
            for j in range(5):
                movz[32 * g + w16_i + j, cls_i] = movv[:, g, j]
        in_maps.append({
            "featt": featt,
            "movz": np.ascontiguousarray(movz.reshape(128, MV_FD)),
            "biasr": bias_rep,
            "brow": np.ascontiguousarray(bias_rep.T.astype(np.float16)),
            "ones": np.ones((1, 512), np.float16),
        })
    return in_maps


def _unshard(res):
    out = np.empty((B, H, W, C), np.float32)
    for core in range(8):
        b, hh = core // 2, core % 2
        r = np.asarray(res.results[core]["outd"]).astype(np.float32)
        r = r.reshape(NG, C, NWT, HS, GW)
        r = r.transpose(3, 2, 0, 4, 1).reshape(HS, W, C)
        out[b, hh * HS:(hh + 1) * HS] = r
    return out


def _run(feat, kernel, bias, **run_kwargs):
    nc = _get_nc()
    in_maps = _prep_inputs(feat, kernel, bias)
    res = run_bass_kernel_spmd(nc, in_maps, core_ids=list(range(8)),
                               **run_kwargs)
    return _unshard(res), res


def kernel(feat, kernel, bias):
    out, _ = _run(np.asarray(feat, np.float32), np.asarray(kernel, np.float32),
                  np.asarray(bias, np.float32))
    return out


# revision 3
# speedup vs baseline: 1.1176x; 1.1176x over previous
"""Per-pixel predicted 5x5 conv (KPN-style) on 8 trn2 cores — banded-matmul PE design.

Sharding: data-parallel over (batch x H-half) = 8 shards of 128 output rows.

All 25 tap-multiplies run on the PE as banded matmuls contracting over w'
(the source column):
    out[h, w, c] = sum_{i,j} kern[h,w,i*5+j] * feat[h+i-2, w+j-2, c]
For one feat row F = h0+rt-2 and one 128-wide w-tile (4 groups of 32), a
single K=116 matmul computes the j-sums for 16 w-offsets x 5 output rows
(h = rt-4..rt) x 4 pixel-groups x 32 channels:
  - stationary lhsT [116, 128]: partition rows [32g, 32g+20) hold
    feat[F, 32g+base+k, c] in a block-diagonal 128-wide layout (g-block at
    columns 32g..32g+32, zeros elsewhere); built by partition-aligned DVE
    copies into a double-buffered generation arena from a per-(wt,half)
    feat upload; the zero regions are memset once and persist.
  - moving rhs [116, 16x5]: kernel taps, row 32g+w16+j of column (w16, hidx)
    holds kern[h, w, i*5+j] with i = 4-hidx; resident class arena
    [128, (cls 32, t 264, hidx 5)] with cls = 2*w16+half, uploaded by 32
    composite-partition DMAs; in-range off-band rows zeroed once (split
    across DVE/ACT/Pool).
  - psum [128=(g,c), (w-off 32, hslot 32)] accumulates each row's 5 i-visits;
    start=True on the first visit (hidx=4) replaces bias seeds.
  - ACT evacuates 16-row batches psum->SBUF fp16 with the channel bias folded
    in via the per-partition activation bias operand.
Output leaves in the native [128=(g,c), wt, h, w-off] layout; the host
unshards.
"""

import sys

for p in ("/opt/pypackages", "/opt/trn_rl_repo"):
    if p not in sys.path:
        sys.path.insert(0, p)

import numpy as np

import concourse.mybir as mybir
from concourse import bacc, tile
from concourse.bass_utils import run_bass_kernel_spmd

B, H, W, C, KK, K = 4, 256, 256, 32, 25, 5
HS = H // 2            # 128 output rows per core
NG, GW = 4, 32         # 4 w-groups of 32 per w-tile
NWT = 2                # two 128-wide w-tiles
NRT = HS + 4           # 132 stationary feat-row tiles (h0-2 .. h0+129)
GEN = 33               # r-slots per stationary generation
NGEN = NRT // GEN      # 4
RING = 32              # psum h-slot ring
EVB = 16               # evac h-batch
NCLS = 32              # moving classes; cls = 16*half + w16
NT = NRT * NWT         # 264 tiles per class
CLS_P = NT * 5         # class free-pitch in elements (1320)
MV_FD = NCLS * CLS_P   # moving arena free elems (42240)
FT_FD = NRT * C        # featT free elems (4224)
GB_FD = GEN * 128      # genbuf free elems (4224)
KDIM = 128             # contraction rows (4 ranges of 20 at 32-aligned bases, zero-padded)

F16 = mybir.dt.float16
F32 = mybir.dt.float32
U32 = mybir.dt.uint32

_NC_CACHE = {}


def _build_nc():
    nc = bacc.Bacc(None, target_bir_lowering=False)
    featt_d = nc.dram_tensor("featt", [4, NG, 20, NRT, C], F16, kind="ExternalInput")
    mov_d = nc.dram_tensor("movz", [128, MV_FD], F16, kind="ExternalInput")
    bias_d = nc.dram_tensor("biasr", [128, 1], F32, kind="ExternalInput")
    brow_d = nc.dram_tensor("brow", [1, 128], F16, kind="ExternalInput")
    ones_d = nc.dram_tensor("ones", [1, 512], F16, kind="ExternalInput")
    out_d = nc.dram_tensor("outd", [128, NWT, HS, GW], F16, kind="ExternalOutput")

    with tile.TileContext(nc) as tc:
        with tc.tile_pool(name="const", bufs=1) as cpool, \
             tc.tile_pool(name="gen", bufs=1) as gpool, \
             tc.tile_pool(name="stg", bufs=1) as spool, \
             tc.tile_pool(name="psum", bufs=1, space="PSUM") as qpool:
            biast = cpool.tile([128, 1], F32, tag="biast")
            brow = cpool.tile([1, 128], F16, tag="brow")
            onest = cpool.tile([1, 512], F16, tag="onest")
            marena = cpool.tile([128, MV_FD], F16, tag="marena")
            featT = []
            for s in range(4):
                t = cpool.tile([128, FT_FD], F16, tag=f"ft{s}", name=f"featT_{s}")
                featT.append(t)
            genbuf = []
            for s in range(4):
                pair = []
                for pb in range(2):
                    t = gpool.tile([128, GB_FD], F16, tag=f"gb{s}_{pb}",
                                   name=f"genbuf_{s}_{pb}")
                    pair.append(t)
                genbuf.append(pair)
            psum = []
            for wt in range(NWT):
                t = qpool.tile([128, GW * RING], F32, tag=f"ps{wt}",
                               name=f"psum_{wt}")
                psum.append(t)
            stage = []
            for sb in range(2):
                t = spool.tile([128, NWT * EVB * GW], F16, tag=f"st{sb}",
                               name=f"stage_{sb}")
                stage.append(t)

            mv_v = marena.rearrange("p (c t h) -> p c t h", c=NCLS, t=NT)

            def zero_u32(eng, reg):
                ru = reg.bitcast(U32)
                if eng == "dve":
                    nc.vector.tensor_scalar_mul(ru, ru, 0.0)
                elif eng == "act":
                    nc.scalar.memzero(reg)
                else:
                    nc.gpsimd.memset(reg, 0.0)

            # ---------- phase 0 ----------
            nc.sync.dma_start(out=biast, in_=bias_d[:, :])
            nc.sync.dma_start(out=brow, in_=brow_d[:, :])
            nc.sync.dma_start(out=onest, in_=ones_d[:, :])
            # feat uploads: compact per-g plain-range DMAs

            def build_gen_s(gi, s):
                pb = gi % 2
                r0 = gi * GEN
                ft_v = featT[s].rearrange("p (r c) -> p r c", r=NRT)
                gb_v = genbuf[s][pb].rearrange("p (r x) -> p r x", r=GEN)
                for g in range(NG):
                    nc.vector.tensor_copy(
                        out=gb_v[32 * g:32 * g + 20, :, GW * g:GW * g + GW],
                        in_=ft_v[32 * g:32 * g + 20, r0:r0 + GEN, :])

            for s in range(4):
                for g in range(NG):
                    dst = featT[s].rearrange("p (r c) -> p r c", r=NRT)[
                        32 * g:32 * g + 20, :, :]
                    nc.sync.dma_start(out=dst, in_=featt_d[s, g])
                zero_u32("dve", genbuf[s][0][:, :])
                build_gen_s(0, s)

            # moving arena: zeros prebuilt on host, 8 t-range chunks so the
            # first iterations only wait for the first chunk
            TQ = GEN * NWT  # 66 tiles per chunk (one generation)
            for q in range(4):
                dst = mv_v[:, :, q * TQ:(q + 1) * TQ, :]
                srcq = mov_d.rearrange("p (c t h) -> p c t h", c=NCLS, t=NT)[
                    :, :, q * TQ:(q + 1) * TQ, :]
                nc.sync.dma_start(out=dst, in_=srcq)

            for s in range(4):
                zero_u32("dve", genbuf[s][1][:, :])
                build_gen_s(1, s)

            # ---------- main loop ----------
            ps_v = [psum[wt].rearrange("p (s w) -> p s w", s=RING)
                    for wt in range(NWT)]
            st_v = [stage[sb].rearrange("p (w h x) -> p w h x", w=NWT, h=EVB)
                    for sb in range(2)]

            def sub_mm(rt, wt, half, hidx):
                s = 2 * wt + half
                gi, slot = rt // GEN, rt % GEN
                lhsT = genbuf[s][gi % 2][0:KDIM, 128 * slot:128 * slot + 128]
                t = 2 * rt + wt
                rhs = mv_v[0:KDIM, 16 * half:16 * half + 16, t, hidx]
                sl = (rt - 4 + hidx) % RING
                out_ap = ps_v[wt][:, sl, 16 * half:16 * half + 16]
                nc.tensor.matmul(out_ap, lhsT, rhs,
                                 start=False, stop=False,
                                 skip_group_check=True, tile_position=(0, 0))

            def seed(hb, wt):
                # bias-seed one full psum bank (16 h-slots) with start=True:
                # start clears has_written bank-wide, so it must cover the
                # bank exactly, before any accumulation into it.
                b0 = ((hb % RING) // EVB) * 512
                nc.tensor.matmul(psum[wt][:, b0:b0 + 512], brow[0:1, :],
                                 onest[0:1, :], start=True, stop=False,
                                 skip_group_check=True, tile_position=(0, 0))

            def evac(hb, wt):
                sb = (hb // EVB) % 2
                sl0 = hb % RING
                src = ps_v[wt][:, sl0:sl0 + EVB, :]
                dst = st_v[sb][:, wt, :, :]
                nc.scalar.copy(out=dst, in_=src)

            def flush(hb):
                sb = (hb // EVB) % 2
                nc.sync.dma_start(
                    out=out_d[:, :, hb:hb + EVB, :], in_=st_v[sb])

            for wt in range(NWT):
                seed(0, wt)
                seed(EVB, wt)
            for rt in range(NRT):
                if rt % GEN == 0 and rt // GEN >= 1 and rt // GEN + 1 < NGEN:
                    for s in range(4):
                        build_gen_s(rt // GEN + 1, s)
                if rt % EVB == 8 and rt + 8 < HS:
                    for wt in range(NWT):
                        seed(rt + 8, wt)
                for wt in range(NWT):
                    for half in range(2):
                        for hidx in range(5):
                            sub_mm(rt, wt, half, hidx)
                hb = rt - (EVB + 3)
                if hb >= 0 and hb % EVB == 0 and hb < HS:
                    for wt in range(NWT):
                        evac(hb, wt)
                    flush(hb)
    if not nc.is_finalized():
        nc.finalize()
    return nc


def _get_nc():
    if "nc" not in _NC_CACHE:
        _NC_CACHE["nc"] = _build_nc()
    return _NC_CACHE["nc"]


def _prep_inputs(feat, kernel, bias):
    feat16 = feat.astype(np.float16)
    k16 = kernel.astype(np.float16)
    in_maps = []
    fp = np.zeros((B, H + 4, W + 4, C), np.float16)
    fp[:, 2:H + 2, 2:W + 2, :] = feat16

    bias_rep = np.tile(bias.astype(np.float32), NG)[:, None]
    bias_rep = np.ascontiguousarray(bias_rep)

    cls_i = np.arange(NCLS)
    half_i = cls_i // 16
    w16_i = cls_i % 16
    g_i = np.arange(NG)
    j_i = np.arange(5)
    t_i = np.arange(NT)
    rt_i = t_i // 2
    wt_i = t_i % 2
    hx_i = np.arange(5)
    W_idx = (128 * wt_i[None, None, :] + 32 * g_i[None, :, None]
             + (16 * half_i + w16_i)[:, None, None])          # [cls, g, t]
    Hrel = rt_i[:, None] - 4 + hx_i[None, :]                   # [t, hidx]
    TAP = (4 - hx_i)[None, :] * 5 + j_i[:, None]               # [j, hidx]
    h_valid = (Hrel >= 0) & (Hrel < HS)

    for core in range(8):
        b, hh = core // 2, core % 2
        h0 = hh * HS

        featt = np.zeros((4, NG, 20, NRT, C), np.float16)
        rows_h = h0 + np.arange(NRT)
        for wt in range(NWT):
            for half in range(2):
                s = 2 * wt + half
                base = 128 * wt + (-2 if half == 0 else 14)
                for g in range(NG):
                    wcols = base + 32 * g + np.arange(20) + 2
                    featt[s, g] = fp[b, rows_h][:, wcols, :].transpose(1, 0, 2)

        kc = k16[b, h0:h0 + HS]
        hcl = np.clip(Hrel, 0, HS - 1)
        movv = kc[hcl[None, None, None, :, :],
                  W_idx[:, :, None, :, None],
                  TAP[None, None, :, None, :]]
        movv = (movv * h_valid[None, None, None, :, :]).astype(np.float16)
        movz = np.zeros((128, NCLS, NT, 5), np.float16)
        for g in range(NG):
            for j in range(5):
                movz[32 * g + w16_i + j, cls_i] = movv[:, g, j]
        in_maps.append({
            "featt": featt,
            "movz": np.ascontiguousarray(movz.reshape(128, MV_FD)),
            "biasr": bias_rep,
            "brow": np.ascontiguousarray(bias_rep.T.astype(np.float16)),
            "ones": np.ones((1, 512), np.float16),
        })
    return in_maps


def _unshard(res):
    out = np.empty((B, H, W, C), np.float32)
    for core in range(8):
        b, hh = core // 2, core % 2
        r = np.asarray(res.results[core]["outd"]).astype(np.float32)
        r = r.reshape(NG, C, NWT, HS, GW)
        r = r.transpose(3, 2, 0, 4, 1).reshape(HS, W, C)
        out[b, hh * HS:(hh + 1) * HS] = r
    return out


def _run(feat, kernel, bias, **run_kwargs):
    nc = _get_nc()
    in_maps = _prep_inputs(feat, kernel, bias)
    res = run_bass_kernel_spmd(nc, in_maps, core_ids=list(range(8)),
                               **run_kwargs)
    return _unshard(res), res


def kernel(feat, kernel, bias):
    out, _ = _run(np.asarray(feat, np.float32), np.asarray(kernel, np.float32),
                  np.asarray(bias, np.float32))
    return out


# revision 4
# speedup vs baseline: 1.1189x; 1.0011x over previous
"""Per-pixel predicted 5x5 conv (KPN-style) on 8 trn2 cores — banded-matmul PE design.

Sharding: data-parallel over (batch x H-half) = 8 shards of 128 output rows.

All 25 tap-multiplies run on the PE as banded matmuls contracting over w'
(the source column):
    out[h, w, c] = sum_{i,j} kern[h,w,i*5+j] * feat[h+i-2, w+j-2, c]
For one feat row F = h0+rt-2 and one 128-wide w-tile (4 groups of 32), a
single K=116 matmul computes the j-sums for 16 w-offsets x 5 output rows
(h = rt-4..rt) x 4 pixel-groups x 32 channels:
  - stationary lhsT [116, 128]: partition rows [32g, 32g+20) hold
    feat[F, 32g+base+k, c] in a block-diagonal 128-wide layout (g-block at
    columns 32g..32g+32, zeros elsewhere); built by partition-aligned DVE
    copies into a double-buffered generation arena from a per-(wt,half)
    feat upload; the zero regions are memset once and persist.
  - moving rhs [116, 16x5]: kernel taps, row 32g+w16+j of column (w16, hidx)
    holds kern[h, w, i*5+j] with i = 4-hidx; resident class arena
    [128, (cls 32, t 264, hidx 5)] with cls = 2*w16+half, uploaded by 32
    composite-partition DMAs; in-range off-band rows zeroed once (split
    across DVE/ACT/Pool).
  - psum [128=(g,c), (w-off 32, hslot 32)] accumulates each row's 5 i-visits;
    start=True on the first visit (hidx=4) replaces bias seeds.
  - ACT evacuates 16-row batches psum->SBUF fp16 with the channel bias folded
    in via the per-partition activation bias operand.
Output leaves in the native [128=(g,c), wt, h, w-off] layout; the host
unshards.
"""

import sys

for p in ("/opt/pypackages", "/opt/trn_rl_repo"):
    if p not in sys.path:
        sys.path.insert(0, p)

import numpy as np

import concourse.mybir as mybir
from concourse import bacc, tile
from concourse.bass_utils import run_bass_kernel_spmd

B, H, W, C, KK, K = 4, 256, 256, 32, 25, 5
HS = H // 2            # 128 output rows per core
NG, GW = 4, 32         # 4 w-groups of 32 per w-tile
NWT = 2                # two 128-wide w-tiles
NRT = HS + 4           # 132 stationary feat-row tiles (h0-2 .. h0+129)
GEN = 33               # r-slots per stationary generation
NGEN = NRT // GEN      # 4
RING = 48              # psum h-slot ring (3 psum banks per w-tile)
EVB = 16               # evac h-batch
NCLS = 32              # moving classes; cls = 16*half + w16
NT = NRT * NWT         # 264 tiles per class
CLS_P = NT * 5         # class free-pitch in elements (1320)
MV_FD = NCLS * CLS_P   # moving arena free elems (42240)
FT_FD = NRT * C        # featT free elems (4224)
GB_FD = GEN * 128      # genbuf free elems (4224)
KDIM = 128             # contraction rows (4 ranges of 20 at 32-aligned bases, zero-padded)

F16 = mybir.dt.float16
F32 = mybir.dt.float32
U32 = mybir.dt.uint32

_NC_CACHE = {}


def _build_nc():
    nc = bacc.Bacc(None, target_bir_lowering=False)
    featt_d = nc.dram_tensor("featt", [4, NG, 20, NRT, C], F16, kind="ExternalInput")
    mov_d = nc.dram_tensor("movz", [128, MV_FD], F16, kind="ExternalInput")
    bias_d = nc.dram_tensor("biasr", [128, 1], F32, kind="ExternalInput")
    brow_d = nc.dram_tensor("brow", [1, 128], F16, kind="ExternalInput")
    ones_d = nc.dram_tensor("ones", [1, 512], F16, kind="ExternalInput")
    out_d = nc.dram_tensor("outd", [128, NWT, HS, GW], F16, kind="ExternalOutput")

    with tile.TileContext(nc) as tc:
        with tc.tile_pool(name="const", bufs=1) as cpool, \
             tc.tile_pool(name="gen", bufs=1) as gpool, \
             tc.tile_pool(name="stg", bufs=1) as spool, \
             tc.tile_pool(name="psum", bufs=1, space="PSUM") as qpool:
            biast = cpool.tile([128, 1], F32, tag="biast")
            brow = cpool.tile([1, 128], F16, tag="brow")
            onest = cpool.tile([1, 512], F16, tag="onest")
            marena = cpool.tile([128, MV_FD], F16, tag="marena")
            featT = []
            for s in range(4):
                t = cpool.tile([128, FT_FD], F16, tag=f"ft{s}", name=f"featT_{s}")
                featT.append(t)
            genbuf = []
            for s in range(4):
                pair = []
                for pb in range(2):
                    t = gpool.tile([128, GB_FD], F16, tag=f"gb{s}_{pb}",
                                   name=f"genbuf_{s}_{pb}")
                    pair.append(t)
                genbuf.append(pair)
            psum = []
            for wt in range(NWT):
                t = qpool.tile([128, GW * RING], F32, tag=f"ps{wt}",
                               name=f"psum_{wt}")
                psum.append(t)
            stage = []
            for sb in range(4):
                t = spool.tile([128, NWT * EVB * GW], F16, tag=f"st{sb}",
                               name=f"stage_{sb}")
                stage.append(t)

            mv_v = marena.rearrange("p (c t h) -> p c t h", c=NCLS, t=NT)

            def zero_u32(eng, reg):
                ru = reg.bitcast(U32)
                if eng == "dve":
                    nc.vector.tensor_scalar_mul(ru, ru, 0.0)
                elif eng == "act":
                    nc.scalar.memzero(reg)
                else:
                    nc.gpsimd.memset(reg, 0.0)

            # ---------- phase 0 ----------
            nc.sync.dma_start(out=biast, in_=bias_d[:, :])
            nc.sync.dma_start(out=brow, in_=brow_d[:, :])
            nc.sync.dma_start(out=onest, in_=ones_d[:, :])
            # feat uploads: compact per-g plain-range DMAs

            def build_gen_s(gi, s):
                pb = gi % 2
                r0 = gi * GEN
                ft_v = featT[s].rearrange("p (r c) -> p r c", r=NRT)
                gb_v = genbuf[s][pb].rearrange("p (r x) -> p r x", r=GEN)
                for g in range(NG):
                    nc.vector.tensor_copy(
                        out=gb_v[32 * g:32 * g + 20, :, GW * g:GW * g + GW],
                        in_=ft_v[32 * g:32 * g + 20, r0:r0 + GEN, :])

            for s in range(4):
                for g in range(NG):
                    dst = featT[s].rearrange("p (r c) -> p r c", r=NRT)[
                        32 * g:32 * g + 20, :, :]
                    nc.sync.dma_start(out=dst, in_=featt_d[s, g])
                zero_u32("dve", genbuf[s][0][:, :])
                build_gen_s(0, s)

            # moving arena: zeros prebuilt on host, 8 t-range chunks so the
            # first iterations only wait for the first chunk
            TQ = GEN * NWT  # 66 tiles per chunk (one generation)
            for q in range(4):
                dst = mv_v[:, :, q * TQ:(q + 1) * TQ, :]
                srcq = mov_d.rearrange("p (c t h) -> p c t h", c=NCLS, t=NT)[
                    :, :, q * TQ:(q + 1) * TQ, :]
                nc.sync.dma_start(out=dst, in_=srcq)

            for s in range(4):
                zero_u32("dve", genbuf[s][1][:, :])
                build_gen_s(1, s)

            # ---------- main loop ----------
            ps_v = [psum[wt].rearrange("p (s w) -> p s w", s=RING)
                    for wt in range(NWT)]
            st_v = [stage[sb].rearrange("p (w h x) -> p w h x", w=NWT, h=EVB)
                    for sb in range(4)]

            def sub_mm(rt, wt, half, hidx):
                s = 2 * wt + half
                gi, slot = rt // GEN, rt % GEN
                lhsT = genbuf[s][gi % 2][0:KDIM, 128 * slot:128 * slot + 128]
                t = 2 * rt + wt
                rhs = mv_v[0:KDIM, 16 * half:16 * half + 16, t, hidx]
                sl = (rt - 4 + hidx) % RING
                out_ap = ps_v[wt][:, sl, 16 * half:16 * half + 16]
                nc.tensor.matmul(out_ap, lhsT, rhs,
                                 start=False, stop=False,
                                 skip_group_check=True, tile_position=(0, 0))

            def seed(hb, wt):
                # bias-seed one full psum bank (16 h-slots) with start=True:
                # start clears has_written bank-wide, so it must cover the
                # bank exactly, before any accumulation into it.
                b0 = ((hb % RING) // EVB) * 512
                nc.tensor.matmul(psum[wt][:, b0:b0 + 512], brow[0:1, :],
                                 onest[0:1, :], start=True, stop=False,
                                 skip_group_check=True, tile_position=(0, 0))

            def evac(hb, wt):
                sb = (hb // EVB) % 4
                sl0 = hb % RING
                src = ps_v[wt][:, sl0:sl0 + EVB, :]
                dst = st_v[sb][:, wt, :, :]
                nc.scalar.copy(out=dst, in_=src)

            def flush(hb):
                sb = (hb // EVB) % 4
                nc.sync.dma_start(
                    out=out_d[:, :, hb:hb + EVB, :], in_=st_v[sb])

            for wt in range(NWT):
                seed(0, wt)
                seed(EVB, wt)
            for rt in range(NRT):
                if rt % GEN == 0 and rt // GEN >= 1 and rt // GEN + 1 < NGEN:
                    for s in range(4):
                        build_gen_s(rt // GEN + 1, s)
                if rt % EVB == 8 and rt + 8 < HS:
                    for wt in range(NWT):
                        seed(rt + 8, wt)
                for wt in range(NWT):
                    for half in range(2):
                        for hidx in range(5):
                            sub_mm(rt, wt, half, hidx)
                hb = rt - (EVB + 3)
                if hb >= 0 and hb % EVB == 0 and hb < HS:
                    for wt in range(NWT):
                        evac(hb, wt)
                    flush(hb)
    if not nc.is_finalized():
        nc.finalize()
    return nc


def _get_nc():
    if "nc" not in _NC_CACHE:
        _NC_CACHE["nc"] = _build_nc()
    return _NC_CACHE["nc"]


def _prep_inputs(feat, kernel, bias):
    feat16 = feat.astype(np.float16)
    k16 = kernel.astype(np.float16)
    in_maps = []
    fp = np.zeros((B, H + 4, W + 4, C), np.float16)
    fp[:, 2:H + 2, 2:W + 2, :] = feat16

    bias_rep = np.tile(bias.astype(np.float32), NG)[:, None]
    bias_rep = np.ascontiguousarray(bias_rep)

    cls_i = np.arange(NCLS)
    half_i = cls_i // 16
    w16_i = cls_i % 16
    g_i = np.arange(NG)
    j_i = np.arange(5)
    t_i = np.arange(NT)
    rt_i = t_i // 2
    wt_i = t_i % 2
    hx_i = np.arange(5)
    W_idx = (128 * wt_i[None, None, :] + 32 * g_i[None, :, None]
             + (16 * half_i + w16_i)[:, None, None])          # [cls, g, t]
    Hrel = rt_i[:, None] - 4 + hx_i[None, :]                   # [t, hidx]
    TAP = (4 - hx_i)[None, :] * 5 + j_i[:, None]               # [j, hidx]
    h_valid = (Hrel >= 0) & (Hrel < HS)

    for core in range(8):
        b, hh = core // 2, core % 2
        h0 = hh * HS

        featt = np.zeros((4, NG, 20, NRT, C), np.float16)
        rows_h = h0 + np.arange(NRT)
        for wt in range(NWT):
            for half in range(2):
                s = 2 * wt + half
                base = 128 * wt + (-2 if half == 0 else 14)
                for g in range(NG):
                    wcols = base + 32 * g + np.arange(20) + 2
                    featt[s, g] = fp[b, rows_h][:, wcols, :].transpose(1, 0, 2)

        kc = k16[b, h0:h0 + HS]
        hcl = np.clip(Hrel, 0, HS - 1)
        movv = kc[hcl[None, None, None, :, :],
                  W_idx[:, :, None, :, None],
                  TAP[None, None, :, None, :]]
        movv = (movv * h_valid[None, None, None, :, :]).astype(np.float16)
        movz = np.zeros((128, NCLS, NT, 5), np.float16)
        for g in range(NG):
            for j in range(5):
                movz[32 * g + w16_i + j, cls_i] = movv[:, g, j]
        in_maps.append({
            "featt": featt,
            "movz": np.ascontiguousarray(movz.reshape(128, MV_FD)),
            "biasr": bias_rep,
            "brow": np.ascontiguousarray(bias_rep.T.astype(np.float16)),
            "ones": np.ones((1, 512), np.float16),
        })
    return in_maps


def _unshard(res):
    out = np.empty((B, H, W, C), np.float32)
    for core in range(8):
        b, hh = core // 2, core % 2
        r = np.asarray(res.results[core]["outd"]).astype(np.float32)
        r = r.reshape(NG, C, NWT, HS, GW)
        r = r.transpose(3, 2, 0, 4, 1).reshape(HS, W, C)
        out[b, hh * HS:(hh + 1) * HS] = r
    return out


def _run(feat, kernel, bias, **run_kwargs):
    nc = _get_nc()
    in_maps = _prep_inputs(feat, kernel, bias)
    res = run_bass_kernel_spmd(nc, in_maps, core_ids=list(range(8)),
                               **run_kwargs)
    return _unshard(res), res


def kernel(feat, kernel, bias):
    out, _ = _run(np.asarray(feat, np.float32), np.asarray(kernel, np.float32),
                  np.asarray(bias, np.float32))
    return out
